# revision 1
# baseline (speedup 1.0000x reference)
"""KNN (K=1, euclidean) Trainium2 kernel.

Strategy
--------
Shard the 4096 y-rows across 8 NeuronCores (512 each); replicate x.
Per core, for each 128-row x tile (32 tiles):
  PSUM[128,512] = sum_k (-2x)^T_k @ y^T_k  (fp16 inputs, fp32 accum, 24 k-tiles)
                + aug matmul adding y^2 (3-way fp16 split rows)
  => t[i,j] = |y_j|^2 - 2 x_i.y_j   (argmin_j t == argmin_j dist, x^2 const per row)
  DVE: tensor_tensor_reduce -> row min (and SBUF copy of t)
       tensor_scalar        -> diff = t - min
       tensor_tensor_reduce -> min_j(diff + j*2^-30)  = eps-packed argmin
       tensor_scalar is_le  -> count of j within BAND of min (near-tie flag)
Host: decodes per-core candidates, recomputes candidate distances exactly in
fp64, resolves flagged near-tie rows with a full fp64 row recompute, applies
sqrt / buffer scatter-update semantics of the reference.
"""

import numpy as np

P = 128          # partitions
KT = 24          # k tiles (3072 / 128)
NJ = 512         # y rows per core
MT = 32          # x tiles (4096 / 128)
NCORES = 8
D = 3072
B = 4096
EPS = 2.0 ** -30
BAND = 0.5
BIG = 1e30

_CACHE = {}


def build_nc(mt=MT):
    import concourse.bacc as bacc
    import concourse.mybir as mybir
    import concourse.tile as tile

    f16 = mybir.dt.float16
    f32 = mybir.dt.float32

    nc = bacc.Bacc("TRN2", target_bir_lowering=False, debug=False)

    xw = nc.dram_tensor("xw", (mt, P, KT, P), f16, kind="ExternalInput")
    yw = nc.dram_tensor("yw", (KT, P, NJ), f16, kind="ExternalInput")
    augw = nc.dram_tensor("augw", (P, P), f16, kind="ExternalInput")
    augy = nc.dram_tensor("augy", (P, NJ), f16, kind="ExternalInput")
    iote = nc.dram_tensor("iote", (P, NJ), f32, kind="ExternalInput")
    res = nc.dram_tensor("res", (P, 3 * mt), f32, kind="ExternalOutput")

    with tile.TileContext(nc) as tc:
        with (
            tc.tile_pool(name="const", bufs=1) as cpool,
            tc.tile_pool(name="xpool", bufs=3) as xpool,
            tc.tile_pool(name="work", bufs=3) as wpool,
            tc.tile_pool(name="resp", bufs=1) as rpool,
            tc.tile_pool(name="psum", bufs=2, space="PSUM") as ppool,
        ):
            y_tiles = []
            for k in range(KT):
                yt = cpool.tile((P, NJ), f16, tag=f"y{k}")
                nc.sync.dma_start(yt[:], yw[k])
                y_tiles.append(yt)
            augw_sb = cpool.tile((P, P), f16)
            nc.sync.dma_start(augw_sb[:], augw[:])
            augy_sb = cpool.tile((P, NJ), f16)
            nc.sync.dma_start(augy_sb[:], augy[:])
            iote_sb = cpool.tile((P, NJ), f32)
            nc.sync.dma_start(iote_sb[:], iote[:])
            res_sb = rpool.tile((P, 3 * mt), f32)

            for m in range(mt):
                x_sb = xpool.tile((P, KT, P), f16, tag="xw")
                nc.sync.dma_start(x_sb[:], xw[m])
                ps = ppool.tile((P, NJ), f32, tag="ps")
                for k in range(KT):
                    nc.tensor.matmul(
                        ps[:], x_sb[:, k, :], y_tiles[k][:],
                        start=(k == 0), stop=False,
                    )
                nc.tensor.matmul(ps[:], augw_sb[:], augy_sb[:],
                                 start=False, stop=True)

                diff = wpool.tile((P, NJ), f32, tag="diff")
                dj = wpool.tile((P, NJ), f32, tag="dj")
                msk = wpool.tile((P, NJ), f32, tag="msk")
                # u = 2x.y - y^2 accumulates in ps; max(u) == -min(t)
                umax = res_sb[:, 3 * m:3 * m + 1]
                nc.vector.tensor_reduce(umax, ps[:],
                                        axis=mybir.AxisListType.X,
                                        op=mybir.AluOpType.max)
                nc.vector.tensor_tensor(
                    out=diff[:], in0=umax.broadcast_to((P, NJ)), in1=ps[:],
                    op=mybir.AluOpType.subtract)
                nc.vector.tensor_tensor(
                    out=dj[:], in0=diff[:], in1=iote_sb[:],
                    op=mybir.AluOpType.add)
                nc.vector.tensor_reduce(res_sb[:, 3 * m + 1:3 * m + 2], dj[:],
                                        axis=mybir.AxisListType.X,
                                        op=mybir.AluOpType.min)
                nc.vector.tensor_scalar(
                    out=msk[:], in0=diff[:], scalar1=float(BAND), scalar2=None,
                    op0=mybir.AluOpType.is_le, op1=mybir.AluOpType.add,
                    accum_out=res_sb[:, 3 * m + 2:3 * m + 3],
                )
            nc.sync.dma_start(res[:], res_sb[:])
    return nc


def make_inputs(x, y):
    """Host-side input prep: per-core in_maps (shared x weights, per-core y)."""
    xs = (2.0 * x.astype(np.float32)).astype(np.float16)
    # xw[mt, p, k, m] = 2x[mt*128+m, k*128+p]
    xw = np.ascontiguousarray(
        xs.reshape(MT, P, KT, P).transpose(0, 3, 2, 1))
    iote = np.broadcast_to(
        (np.arange(NJ, dtype=np.float64) * EPS).astype(np.float32), (P, NJ)
    ).copy()
    augw = np.zeros((P, P), np.float16)
    augw[0:3, :] = 1.0

    y64 = y.astype(np.float64)
    y2g = np.sum(y64 * y64, axis=1)  # fp64 row norms of full y

    in_maps = []
    for c in range(NCORES):
        yc = y[c * NJ:(c + 1) * NJ].astype(np.float16)
        # yw[k, p, n] = y_c[n, k*128+p]  (each k slice contiguous)
        yw = np.ascontiguousarray(yc.reshape(NJ, KT, P).transpose(1, 2, 0))
        y2c = -y2g[c * NJ:(c + 1) * NJ]  # negated: u = 2x.y - y^2
        s1 = y2c.astype(np.float16)
        r1 = y2c - s1.astype(np.float64)
        s2 = r1.astype(np.float16)
        s3 = (r1 - s2.astype(np.float64)).astype(np.float16)
        augy = np.zeros((P, NJ), np.float16)
        augy[0] = s1
        augy[1] = s2
        augy[2] = s3
        in_maps.append({"xw": xw, "yw": yw, "augw": augw,
                        "augy": augy, "iote": iote})
    return in_maps, y2g


def decode_core(res_c, mt=MT):
    """res_c [128, 3*mt] -> (tmin[B], jloc[B], cnt[B], anom[B]) in x-row order."""
    tmin = -res_c[:, 0::3].T.reshape(-1).astype(np.float64)  # t = -u
    jp = res_c[:, 1::3].T.reshape(-1).astype(np.float64)
    cnt = res_c[:, 2::3].T.reshape(-1).astype(np.float64)
    jf = jp / EPS
    jloc = np.rint(jf).astype(np.int64)
    anom = (np.abs(jf - jloc) > 1e-3) | (jloc < 0) | (jloc >= NJ)
    jloc = np.clip(jloc, 0, NJ - 1)
    return tmin, jloc, cnt, anom


def postprocess(results, x, y, y2g, min_dists, nn_indices,
                x_idx_start, y_idx_start):
    nb = x.shape[0]
    x64 = x.astype(np.float64)
    y64 = y.astype(np.float64)
    x2 = np.sum(x64 * x64, axis=1)

    tmins = np.empty((NCORES, nb))
    jglob = np.empty((NCORES, nb), np.int64)
    cnts = np.empty((NCORES, nb))
    anoms = np.zeros(nb, bool)
    for c in range(NCORES):
        tm, jl, cn, an = decode_core(np.asarray(results[c]["res"]))
        tmins[c] = tm
        jglob[c] = c * NJ + jl
        cnts[c] = cn
        anoms |= an

    # exact fp64 t for every per-core candidate
    tex = np.empty((NCORES, nb))
    for c in range(NCORES):
        yj = y64[jglob[c]]
        tex[c] = y2g[jglob[c]] - 2.0 * np.einsum("ij,ij->i", x64, yj)

    order = np.argsort(tex, axis=0, kind="stable")
    bc = order[0]
    rows = np.arange(nb)
    best = tex[bc, rows]
    second = tex[order[1], rows]
    jbest = jglob[bc, rows]

    # exact cross-core tie on best value -> pick smallest j (first occurrence)
    tie = np.abs(tex - best[None, :]) <= 0.0
    jtie = np.where(tie, jglob, np.iinfo(np.int64).max)
    jbest = jtie.min(axis=0)

    flag = anoms.copy()
    flag |= cnts[bc, rows] > 1                       # winner core has near-tie
    flag |= (second - best) <= 1e-3                  # cross-core near-tie
    flag |= np.any((cnts > 1) & (tex <= best[None, :] + BAND + 0.1), axis=0)

    frows = np.where(flag)[0]
    if frows.size:
        CH = 256
        for s in range(0, frows.size, CH):
            rr = frows[s:s + CH]
            tall = y2g[None, :] - 2.0 * (x64[rr] @ y64.T)
            jt = np.argmin(tall, axis=1)
            best[rr] = tall[np.arange(rr.size), jt]
            jbest[rr] = jt

    d2 = x2 + best
    new_min = np.sqrt(np.maximum(d2, 0.0)).astype(np.float32)

    md = np.array(min_dists, dtype=np.float32, copy=True)
    ni = np.array(nn_indices, dtype=np.int32, copy=True)
    n = md.shape[0]
    s = int(np.asarray(x_idx_start))
    s = max(0, min(s, n - nb))  # dynamic_update_slice clamp semantics
    md[s:s + nb] = np.minimum(new_min, md[s:s + nb])
    ni[s:s + nb] = (jbest.astype(np.int64)
                    + int(np.asarray(y_idx_start))).astype(np.int32)
    return md, ni


def _get_nc():
    if "nc" not in _CACHE:
        nc = build_nc()
        nc.compile()
        _CACHE["nc"] = nc
    return _CACHE["nc"]


def run_device(in_maps, trace=False, **kw):
    from concourse.bass_utils import run_bass_kernel_spmd
    nc = _get_nc()
    return run_bass_kernel_spmd(nc, in_maps, list(range(NCORES)),
                                trace=trace, **kw)


def kernel(x, y, min_dists, nn_indices, x_idx_start, y_idx_start):
    x = np.asarray(x)
    y = np.asarray(y)
    in_maps, y2g = make_inputs(x, y)
    br = run_device(in_maps, trace=False)
    return postprocess(br.results, x, y, y2g, min_dists, nn_indices,
                       x_idx_start, y_idx_start)



# revision 5
# speedup vs baseline: 1.8348x; 1.8348x over previous
"""KNN (K=1, euclidean) Trainium2 kernel — fp8 DoubleRow edition.

Strategy
--------
Shard the 4096 y-rows across 8 NeuronCores (512 each); replicate x.
Per core, for each 128-row x tile (32 tiles):
  PSUM[128,512] = sum_k q8(2x)^T_k @ q8(y)^T_k   (fp8 e4m3 DoubleRow
                  matmuls, 2 k-tiles per instruction, 2x PE rate, fp32
                  accum) + aug matmul adding -|y|^2 (5 power-of-2-scaled
                  fp8 channels, residual < 0.05)
  => ps[i,j] = u_ij ~= 2 x_i.y_j - |y_j|^2 ; argmax_j u == argmin_j dist
  DVE (4 passes):
    tensor_reduce(max)           -> umax (row max of ps; -umax = t_min)
    scalar_tensor_tensor         -> dj2 = (ps - umax) - j*2^-10
    tensor_reduce(max)           -> pj = max_j dj2 = -argmax_j * 2^-10
    tensor_scalar(is_ge, accum)  -> cnt = #{j: dj2_j >= -BAND} near-ties
Host: decodes per-core candidates (j = -pj*2^10), recomputes candidate
distances exactly in fp64, and for rows whose winner has near-ties resolves
exactly with per-suspect-core fp64 GEMMs; applies sqrt / buffer
scatter-update semantics of the reference.
"""

import numpy as np
import ml_dtypes

P = 128          # partitions
KT = 24          # k tiles (3072 / 128)
NJ = 512         # y rows per core
MT = 32          # x tiles (4096 / 128)
NCORES = 8
D = 3072
B = 4096
EPS = 2.0 ** -10   # argmax packing step (survives fp32 ulp at |u|~3500)
BAND = 35.0        # near-tie band in u units (fp8 err std ~4.2, 2 pairs)
F8 = ml_dtypes.float8_e4m3

_CACHE = {}


def build_nc(mt=MT):
    import concourse.bacc as bacc
    import concourse.mybir as mybir
    import concourse.tile as tile

    f8 = mybir.dt.float8e4
    f32 = mybir.dt.float32
    bf16 = mybir.dt.bfloat16
    DR = mybir.MatmulPerfMode.DoubleRow

    nc = bacc.Bacc("TRN2", target_bir_lowering=False, debug=False)

    xw = nc.dram_tensor("xw", (mt, P, KT, P), f8, kind="ExternalInput")
    yw = nc.dram_tensor("yw", (P, KT, NJ), f8, kind="ExternalInput")
    augw = nc.dram_tensor("augw", (P, 2, P), f8, kind="ExternalInput")
    augy = nc.dram_tensor("augy", (P, 2, NJ), f8, kind="ExternalInput")
    iote = nc.dram_tensor("iote", (P, NJ), f32, kind="ExternalInput")
    res = nc.dram_tensor("res", (P, 3 * mt), f32, kind="ExternalOutput")

    with tile.TileContext(nc) as tc:
        with (
            tc.tile_pool(name="const", bufs=1) as cpool,
            tc.tile_pool(name="xpool", bufs=4) as xpool,
            tc.tile_pool(name="work", bufs=3) as wpool,
            tc.tile_pool(name="mask", bufs=2) as mpool,
            tc.tile_pool(name="resp", bufs=1) as rpool,
            tc.tile_pool(name="psum", bufs=3, space="PSUM") as ppool,
        ):
            y_sb = cpool.tile((P, KT, NJ), f8, tag="y")
            nc.sync.dma_start(y_sb[:], yw[:])
            augw_sb = cpool.tile((P, 2, P), f8)
            nc.sync.dma_start(augw_sb[:], augw[:])
            augy_sb = cpool.tile((P, 2, NJ), f8)
            nc.sync.dma_start(augy_sb[:], augy[:])
            iote_sb = cpool.tile((P, NJ), f32)
            nc.sync.dma_start(iote_sb[:], iote[:])
            res_sb = rpool.tile((P, 3 * mt), f32)

            for m in range(mt):
                x_sb = xpool.tile((P, KT, P), f8, tag="xw")
                nc.sync.dma_start(x_sb[:], xw[m])
                ps = ppool.tile((P, NJ), f32, tag="ps")
                for kk in range(KT // 2):
                    nc.tensor.matmul(
                        ps[:],
                        x_sb[:, 2 * kk:2 * kk + 2, :],
                        y_sb[:, 2 * kk:2 * kk + 2, :],
                        start=(kk == 0), stop=False, perf_mode=DR,
                    )
                nc.tensor.matmul(ps[:], augw_sb[:], augy_sb[:],
                                 start=False, stop=True, perf_mode=DR)

                umax = res_sb[:, 3 * m:3 * m + 1]
                nc.vector.tensor_reduce(umax, ps[:],
                                        axis=mybir.AxisListType.X,
                                        op=mybir.AluOpType.max)
                # dj2 = (ps - umax) - j*EPS; row max = -argmax*EPS
                # (exact at the winner: ps - umax == 0 there)
                dj2 = wpool.tile((P, NJ), f32, tag="dj2")
                nc.vector.scalar_tensor_tensor(
                    out=dj2[:], in0=ps[:], scalar=umax, in1=iote_sb[:],
                    op0=mybir.AluOpType.subtract, op1=mybir.AluOpType.subtract,
                )
                nc.vector.tensor_reduce(res_sb[:, 3 * m + 1:3 * m + 2], dj2[:],
                                        axis=mybir.AxisListType.X,
                                        op=mybir.AluOpType.max)
                msk = mpool.tile((P, NJ), bf16, tag="msk")
                nc.vector.tensor_scalar(
                    out=msk[:], in0=dj2[:], scalar1=float(-BAND), scalar2=None,
                    op0=mybir.AluOpType.is_ge, op1=mybir.AluOpType.add,
                    accum_out=res_sb[:, 3 * m + 2:3 * m + 3],
                )
            nc.sync.dma_start(res[:], res_sb[:])
    return nc


def _fp8(a):
    return np.asarray(a, np.float32).astype(F8)


def _y2_channels(neg_y2):
    """Split -|y|^2 (fp64, ~[-3500,-2500]) into 5 fp8 channels with exact
    power-of-2 weights so that sum_r w_r * fp8(ch_r) ~= -|y|^2 (|res|<0.05)."""
    ws = [64.0, 8.0, 1.0, 2.0 ** -3, 2.0 ** -6]
    r = neg_y2.copy()
    chans = []
    for w in ws:
        a8 = _fp8(r / w)
        chans.append(a8)
        r = r - w * a8.astype(np.float64)
    return ws, chans, r


def make_inputs(x, y):
    """Host-side input prep: per-core in_maps (shared x weights, per-core y)."""
    xs = _fp8(2.0 * np.asarray(x, np.float32))
    # xw[mt, p, k, m] = q8(2x)[mt*128+m, k*128+p]
    xw = np.ascontiguousarray(
        xs.reshape(MT, P, KT, P).transpose(0, 3, 2, 1))
    iote = np.broadcast_to(
        (np.arange(NJ, dtype=np.float64) * EPS).astype(np.float32), (P, NJ)
    ).copy()

    y64 = np.asarray(y).astype(np.float64)
    y2g = np.sum(y64 * y64, axis=1)  # fp64 row norms of full y

    in_maps = []
    for c in range(NCORES):
        yc8 = _fp8(y[c * NJ:(c + 1) * NJ])
        # yw[p, k, n] = q8(y_c)[n, k*128+p]
        yw = np.ascontiguousarray(yc8.reshape(NJ, KT, P).transpose(2, 1, 0))
        ws, chans, rres = _y2_channels(-y2g[c * NJ:(c + 1) * NJ])
        assert np.abs(rres).max() < 0.05, np.abs(rres).max()
        augw = np.zeros((P, 2, P), F8)
        augy = np.zeros((P, 2, NJ), F8)
        for r, (w, ch) in enumerate(zip(ws, chans)):
            augw[r, 0, :] = w
            augy[r, 0, :] = ch
        in_maps.append({"xw": xw, "yw": yw, "augw": augw,
                        "augy": augy, "iote": iote})
    return in_maps, y2g


def decode_core(res_c, mt=MT):
    """res_c [128, 3*mt] -> (tmin[B], jloc[B], cnt[B], anom[B]) in x-row order."""
    umax = res_c[:, 0::3].T.reshape(-1).astype(np.float64)
    pj = res_c[:, 1::3].T.reshape(-1).astype(np.float64)
    cnt = res_c[:, 2::3].T.reshape(-1).astype(np.float64)
    tmin = -umax                       # t = -u
    jf = -pj / EPS
    jloc = np.rint(jf).astype(np.int64)
    anom = (np.abs(jf - jloc) > 0.35) | (jloc < 0) | (jloc >= NJ)
    jloc = np.clip(jloc, 0, NJ - 1)
    return tmin, jloc, cnt, anom


def postprocess(results, x, y, y2g, min_dists, nn_indices,
                x_idx_start, y_idx_start):
    nb = x.shape[0]
    x64 = np.asarray(x).astype(np.float64)
    y64 = np.asarray(y).astype(np.float64)
    x2 = np.sum(x64 * x64, axis=1)

    tmins = np.empty((NCORES, nb))
    jglob = np.empty((NCORES, nb), np.int64)
    cnts = np.empty((NCORES, nb))
    anoms = np.zeros(nb, bool)
    for c in range(NCORES):
        tm, jl, cn, an = decode_core(np.asarray(results[c]["res"]))
        tmins[c] = tm
        jglob[c] = c * NJ + jl
        cnts[c] = cn
        anoms |= an

    # exact fp64 t for every per-core candidate
    tex = np.empty((NCORES, nb))
    for c in range(NCORES):
        yj = y64[jglob[c]]
        tex[c] = y2g[jglob[c]] - 2.0 * np.einsum("ij,ij->i", x64, yj)

    best = tex.min(axis=0)
    # exact cross-core tie on best value -> pick smallest j
    tie = tex <= best[None, :]
    jtie = np.where(tie, jglob, np.iinfo(np.int64).max)
    jbest = jtie.min(axis=0)

    # suspect cores: near-tie inside the core AND device min close to best.
    # tmins (device) is used, not tex: decode aliasing (two dev values within
    # 512*EPS) can make the candidate j meaningless, but tmin_dev is sound
    # (tmin_dev(c*) <= truemin + E1 <= best + E1, single-pair error bound).
    sus = (cnts >= 1.5) & (tmins <= best[None, :] + BAND)
    flag = sus.any(axis=0) & ~anoms

    # resolve flagged rows with per-core fp64 GEMMs over suspect cores only
    if flag.any():
        for c in range(NCORES):
            rows = np.where(flag & sus[c])[0]
            if not rows.size:
                continue
            yc = y64[c * NJ:(c + 1) * NJ]
            tall = y2g[None, c * NJ:(c + 1) * NJ] - 2.0 * (x64[rows] @ yc.T)
            jt = np.argmin(tall, axis=1)           # first occurrence = min j
            tv = tall[np.arange(rows.size), jt]
            jg = c * NJ + jt
            better = (tv < best[rows]) | ((tv == best[rows]) & (jg < jbest[rows]))
            upd = rows[better]
            best[upd] = tv[better]
            jbest[upd] = jg[better]

    # anomalous rows (decode failure): full-row exact recompute
    frows = np.where(anoms)[0]
    if frows.size:
        CH = 256
        for s in range(0, frows.size, CH):
            rr = frows[s:s + CH]
            tall = y2g[None, :] - 2.0 * (x64[rr] @ y64.T)
            jt = np.argmin(tall, axis=1)
            best[rr] = tall[np.arange(rr.size), jt]
            jbest[rr] = jt

    d2 = x2 + best
    new_min = np.sqrt(np.maximum(d2, 0.0)).astype(np.float32)

    md = np.array(min_dists, dtype=np.float32, copy=True)
    ni = np.array(nn_indices, dtype=np.int32, copy=True)
    n = md.shape[0]
    s = int(np.asarray(x_idx_start))
    s = max(0, min(s, n - nb))  # dynamic_update_slice clamp semantics
    md[s:s + nb] = np.minimum(new_min, md[s:s + nb])
    ni[s:s + nb] = (jbest.astype(np.int64)
                    + int(np.asarray(y_idx_start))).astype(np.int32)
    return md, ni


def _get_nc():
    if "nc" not in _CACHE:
        nc = build_nc()
        nc.compile()
        _CACHE["nc"] = nc
    return _CACHE["nc"]


def run_device(in_maps, trace=False, **kw):
    from concourse.bass_utils import run_bass_kernel_spmd
    nc = _get_nc()
    return run_bass_kernel_spmd(nc, in_maps, list(range(NCORES)),
                                trace=trace, **kw)


def kernel(x, y, min_dists, nn_indices, x_idx_start, y_idx_start):
    x = np.asarray(x)
    y = np.asarray(y)
    in_maps, y2g = make_inputs(x, y)
    br = run_device(in_maps, trace=False)
    return postprocess(br.results, x, y, y2g, min_dists, nn_indices,
                       x_idx_start, y_idx_start)


# revision 10
# speedup vs baseline: 1.8459x; 1.0060x over previous
"""KNN (K=1, euclidean) Trainium2 kernel — fp8 DoubleRow edition.

Strategy
--------
Shard the 4096 y-rows across 8 NeuronCores (512 each); replicate x.
Per core, for each 128-row x tile (32 tiles):
  PSUM[128,512] = sum_k q8(2x)^T_k @ q8(y)^T_k   (fp8 e4m3 DoubleRow
                  matmuls, 2 k-tiles per instruction, 2x PE rate, fp32
                  accum) + aug matmul adding -|y|^2 (5 power-of-2-scaled
                  fp8 channels, residual < 0.05)
  => ps[i,j] = u_ij ~= 2 x_i.y_j - |y_j|^2 ; argmax_j u == argmin_j dist
  DVE (3 passes):
    tensor_reduce(max)           -> umax (row max of ps; -umax = t_min)
    scalar_tensor_tensor         -> dj2 = (ps - umax) - j*2^-10
    tensor_reduce(max)           -> pj = max_j dj2 = -argmax_j * 2^-10
  ACT (near-tie count, off the DVE critical path):
    activation(Copy)             -> bias = BAND - umax
    activation(Sign, accum)      -> craw = sum_j sign(ps_j - umax + BAND)
Host: decodes per-core candidates (j = -pj*2^10), recomputes candidate
distances exactly in fp64, and for rows whose winner has near-ties resolves
exactly with per-suspect-core fp64 GEMMs; applies sqrt / buffer
scatter-update semantics of the reference.
"""

import numpy as np
import ml_dtypes

P = 128          # partitions
KT = 24          # k tiles (3072 / 128)
NJ = 512         # y rows per core
MT = 32          # x tiles (4096 / 128)
NCORES = 8
D = 3072
B = 4096
EPS = 2.0 ** -10   # argmax packing step (survives fp32 ulp at |u|~3500)
BAND = 35.0        # near-tie band in u units (fp8 err std ~4.2, 2 pairs)
F8 = ml_dtypes.float8_e4m3

_CACHE = {}


def build_nc(mt=MT):
    import concourse.bacc as bacc
    import concourse.mybir as mybir
    import concourse.tile as tile

    f8 = mybir.dt.float8e4
    f32 = mybir.dt.float32
    bf16 = mybir.dt.bfloat16
    DR = mybir.MatmulPerfMode.DoubleRow

    nc = bacc.Bacc("TRN2", target_bir_lowering=False, debug=False)

    xw = nc.dram_tensor("xw", (mt, P, KT, P), f8, kind="ExternalInput")
    yw = nc.dram_tensor("yw", (P, KT, NJ), f8, kind="ExternalInput")
    augw = nc.dram_tensor("augw", (P, 2, P), f8, kind="ExternalInput")
    augy = nc.dram_tensor("augy", (P, 2, NJ), f8, kind="ExternalInput")
    iote = nc.dram_tensor("iote", (P, NJ), f32, kind="ExternalInput")
    res = nc.dram_tensor("res", (P, 3 * mt), f32, kind="ExternalOutput")

    with tile.TileContext(nc) as tc:
        with (
            tc.tile_pool(name="const", bufs=1) as cpool,
            tc.tile_pool(name="xpool", bufs=4) as xpool,
            tc.tile_pool(name="work", bufs=3) as wpool,
            tc.tile_pool(name="mask", bufs=2) as mpool,
            tc.tile_pool(name="bias", bufs=2) as bpool,
            tc.tile_pool(name="resp", bufs=1) as rpool,
            tc.tile_pool(name="psum", bufs=4, space="PSUM") as ppool,
        ):
            y_sb = cpool.tile((P, KT, NJ), f8, tag="y")
            nc.sync.dma_start(y_sb[:], yw[:])
            augw_sb = cpool.tile((P, 2, P), f8)
            nc.sync.dma_start(augw_sb[:], augw[:])
            augy_sb = cpool.tile((P, 2, NJ), f8)
            nc.sync.dma_start(augy_sb[:], augy[:])
            iote_sb = cpool.tile((P, NJ), f32)
            nc.sync.dma_start(iote_sb[:], iote[:])
            res_sb = rpool.tile((P, 3 * mt), f32)

            for m in range(mt):
                x_sb = xpool.tile((P, KT, P), f8, tag="xw")
                nc.sync.dma_start(x_sb[:], xw[m])
                ps = ppool.tile((P, NJ), f32, tag="ps")
                for kk in range(KT // 2):
                    nc.tensor.matmul(
                        ps[:],
                        x_sb[:, 2 * kk:2 * kk + 2, :],
                        y_sb[:, 2 * kk:2 * kk + 2, :],
                        start=(kk == 0), stop=False, perf_mode=DR,
                    )
                nc.tensor.matmul(ps[:], augw_sb[:], augy_sb[:],
                                 start=False, stop=True, perf_mode=DR)

                umax = res_sb[:, 3 * m:3 * m + 1]
                nc.vector.tensor_reduce(umax, ps[:],
                                        axis=mybir.AxisListType.X,
                                        op=mybir.AluOpType.max)
                # dj2 = (ps - umax) - j*EPS; row max = -argmax*EPS
                # (exact at the winner: ps - umax == 0 there)
                dj2 = wpool.tile((P, NJ), f32, tag="dj2")
                nc.vector.scalar_tensor_tensor(
                    out=dj2[:], in0=ps[:], scalar=umax, in1=iote_sb[:],
                    op0=mybir.AluOpType.subtract, op1=mybir.AluOpType.subtract,
                )
                nc.vector.tensor_reduce(res_sb[:, 3 * m + 1:3 * m + 2], dj2[:],
                                        axis=mybir.AxisListType.X,
                                        op=mybir.AluOpType.max)
                # near-tie count on the scalar engine (DVE stays 3 passes)
                bias_c = bpool.tile((P, 1), f32, tag="bias")
                nc.scalar.activation(
                    bias_c[:], umax,
                    mybir.ActivationFunctionType.Copy,
                    bias=float(BAND), scale=-1.0,
                )
                msk = mpool.tile((P, NJ), bf16, tag="msk")
                nc.scalar.activation(
                    msk[:], ps[:],
                    mybir.ActivationFunctionType.Sign,
                    bias=bias_c[:], scale=1.0,
                    accum_out=res_sb[:, 3 * m + 2:3 * m + 3],
                )
            nc.sync.dma_start(res[:], res_sb[:])
    return nc


def _fp8(a):
    return np.asarray(a, np.float32).astype(F8)


def _y2_channels(neg_y2):
    """Split -|y|^2 (fp64, ~[-3500,-2500]) into 5 fp8 channels with exact
    power-of-2 weights so that sum_r w_r * fp8(ch_r) ~= -|y|^2 (|res|<0.05)."""
    ws = [64.0, 8.0, 1.0, 2.0 ** -3, 2.0 ** -6]
    r = neg_y2.copy()
    chans = []
    for w in ws:
        a8 = _fp8(r / w)
        chans.append(a8)
        r = r - w * a8.astype(np.float64)
    return ws, chans, r


def make_inputs(x, y):
    """Host-side input prep: per-core in_maps (shared x weights, per-core y)."""
    xs = _fp8(2.0 * np.asarray(x, np.float32))
    # xw[mt, p, k, m] = q8(2x)[mt*128+m, k*128+p]
    xw = np.ascontiguousarray(
        xs.reshape(MT, P, KT, P).transpose(0, 3, 2, 1))
    iote = np.broadcast_to(
        (np.arange(NJ, dtype=np.float64) * EPS).astype(np.float32), (P, NJ)
    ).copy()

    y64 = np.asarray(y).astype(np.float64)
    y2g = np.sum(y64 * y64, axis=1)  # fp64 row norms of full y

    in_maps = []
    for c in range(NCORES):
        yc8 = _fp8(y[c * NJ:(c + 1) * NJ])
        # yw[p, k, n] = q8(y_c)[n, k*128+p]
        yw = np.ascontiguousarray(yc8.reshape(NJ, KT, P).transpose(2, 1, 0))
        ws, chans, rres = _y2_channels(-y2g[c * NJ:(c + 1) * NJ])
        assert np.abs(rres).max() < 0.05, np.abs(rres).max()
        augw = np.zeros((P, 2, P), F8)
        augy = np.zeros((P, 2, NJ), F8)
        for r, (w, ch) in enumerate(zip(ws, chans)):
            augw[r, 0, :] = w
            augy[r, 0, :] = ch
        in_maps.append({"xw": xw, "yw": yw, "augw": augw,
                        "augy": augy, "iote": iote})
    return in_maps, y2g


def decode_core(res_c, mt=MT):
    """res_c [128, 3*mt] -> (tmin[B], jloc[B], cnt[B], anom[B]) in x-row order."""
    umax = res_c[:, 0::3].T.reshape(-1).astype(np.float64)
    pj = res_c[:, 1::3].T.reshape(-1).astype(np.float64)
    craw = res_c[:, 2::3].T.reshape(-1).astype(np.float64)
    cnt = (craw + NJ) / 2.0            # sign-sum -> #{>} + #{=}/2
    tmin = -umax                       # t = -u
    jf = -pj / EPS
    jloc = np.rint(jf).astype(np.int64)
    anom = (np.abs(jf - jloc) > 0.35) | (jloc < 0) | (jloc >= NJ)
    jloc = np.clip(jloc, 0, NJ - 1)
    return tmin, jloc, cnt, anom


def postprocess(results, x, y, y2g, min_dists, nn_indices,
                x_idx_start, y_idx_start):
    nb = x.shape[0]
    x64 = np.asarray(x).astype(np.float64)
    y64 = np.asarray(y).astype(np.float64)
    x2 = np.sum(x64 * x64, axis=1)

    tmins = np.empty((NCORES, nb))
    jglob = np.empty((NCORES, nb), np.int64)
    cnts = np.empty((NCORES, nb))
    anoms = np.zeros(nb, bool)
    for c in range(NCORES):
        tm, jl, cn, an = decode_core(np.asarray(results[c]["res"]))
        tmins[c] = tm
        jglob[c] = c * NJ + jl
        cnts[c] = cn
        anoms |= an

    # exact fp64 t for every per-core candidate
    tex = np.empty((NCORES, nb))
    for c in range(NCORES):
        yj = y64[jglob[c]]
        tex[c] = y2g[jglob[c]] - 2.0 * np.einsum("ij,ij->i", x64, yj)

    best = tex.min(axis=0)
    # exact cross-core tie on best value -> pick smallest j
    tie = tex <= best[None, :]
    jtie = np.where(tie, jglob, np.iinfo(np.int64).max)
    jbest = jtie.min(axis=0)

    # suspect cores: near-tie inside the core AND device min close to best.
    # tmins (device) is used, not tex: decode aliasing (two dev values within
    # 512*EPS) can make the candidate j meaningless, but tmin_dev is sound
    # (tmin_dev(c*) <= truemin + E1 <= best + E1, single-pair error bound).
    sus = (cnts >= 1.4) & (tmins <= best[None, :] + BAND)
    flag = sus.any(axis=0) & ~anoms

    # resolve flagged rows with per-core fp64 GEMMs over suspect cores only
    if flag.any():
        for c in range(NCORES):
            rows = np.where(flag & sus[c])[0]
            if not rows.size:
                continue
            yc = y64[c * NJ:(c + 1) * NJ]
            tall = y2g[None, c * NJ:(c + 1) * NJ] - 2.0 * (x64[rows] @ yc.T)
            jt = np.argmin(tall, axis=1)           # first occurrence = min j
            tv = tall[np.arange(rows.size), jt]
            jg = c * NJ + jt
            better = (tv < best[rows]) | ((tv == best[rows]) & (jg < jbest[rows]))
            upd = rows[better]
            best[upd] = tv[better]
            jbest[upd] = jg[better]

    # anomalous rows (decode failure): full-row exact recompute
    frows = np.where(anoms)[0]
    if frows.size:
        CH = 256
        for s in range(0, frows.size, CH):
            rr = frows[s:s + CH]
            tall = y2g[None, :] - 2.0 * (x64[rr] @ y64.T)
            jt = np.argmin(tall, axis=1)
            best[rr] = tall[np.arange(rr.size), jt]
            jbest[rr] = jt

    d2 = x2 + best
    new_min = np.sqrt(np.maximum(d2, 0.0)).astype(np.float32)

    md = np.array(min_dists, dtype=np.float32, copy=True)
    ni = np.array(nn_indices, dtype=np.int32, copy=True)
    n = md.shape[0]
    s = int(np.asarray(x_idx_start))
    s = max(0, min(s, n - nb))  # dynamic_update_slice clamp semantics
    md[s:s + nb] = np.minimum(new_min, md[s:s + nb])
    ni[s:s + nb] = (jbest.astype(np.int64)
                    + int(np.asarray(y_idx_start))).astype(np.int32)
    return md, ni


def _get_nc():
    if "nc" not in _CACHE:
        nc = build_nc()
        nc.compile()
        _CACHE["nc"] = nc
    return _CACHE["nc"]


def run_device(in_maps, trace=False, **kw):
    from concourse.bass_utils import run_bass_kernel_spmd
    nc = _get_nc()
    return run_bass_kernel_spmd(nc, in_maps, list(range(NCORES)),
                                trace=trace, **kw)


def kernel(x, y, min_dists, nn_indices, x_idx_start, y_idx_start):
    x = np.asarray(x)
    y = np.asarray(y)
    in_maps, y2g = make_inputs(x, y)
    br = run_device(in_maps, trace=False)
    return postprocess(br.results, x, y, y2g, min_dists, nn_indices,
                       x_idx_start, y_idx_start)
